# revision 58
# baseline (speedup 1.0000x reference)
"""Masked multi-head attention (B=2, S=2048, E=1024, H=16, D=64) on 8 TRN2 cores.

Sharding: each core owns 2 heads (of 16) for BOTH batches.
  - QKV projections computed per-core for its 2 heads (transposed layouts).
    Batch-0 projections are software-pipelined INTO batch-0 attention (unit
    (0,qb) only needs seq-block qb), so attention starts right after the
    first 1.3MB of DMA instead of waiting for the full 8MB x load; batch-1
    projection blocks are spread across both batches' attention units.
  - Attention: flash-style with transposed scores (scoresT[k, q] tiles),
    software-pipelined at k-tile granularity (score matmuls of k-tile i+1
    emitted before the PV matmuls of k-tile i). Unsafe softmax (no max
    subtraction); denominator via a leading ones-column in the PV matmul;
    causal mask applied only on the [128,128] diagonal square; score
    matmuls AND exp AND PV all trimmed to the valid q-range of diagonal
    k-tiles. The per-unit softmax normalization broadcasts the reciprocal
    across partitions with a tiny contraction-1 PE matmul against a ones
    row (NOT gpsimd partition_broadcast: gpsimd issues the collectives and
    BLOCKS until they complete, so any gpsimd work queued after a trigger
    would stall the pipeline for the whole transfer). Each unit's finalize
    is emitted inside the next unit so its reciprocal chain hides under
    compute.
  - Exchange head-parallel -> slot-parallel via TWO AllToAlls into SEPARATE
    dram buffers: slots 0-3 (batch 0) exchanged right after batch-0
    attention (hidden under batch-1 compute); slots 4-7 at the end.
  - Output projection runs TWICE: pass 1 reads the batch-0 exchange buffer
    and runs DURING the final AllToAll's transfer window (the PE is
    otherwise idle there); pass 2 reads the batch-1 buffer after it lands.
    Pass g's result is valid on cores with pid//4 == g; both passes write
    their slice of outT[2, E, 512] and the host picks slice c//4 per core.
    Exchange buffers are staged to SBUF with one strided DMA each: the
    pass-1 stage is prefetched once the first AllToAll completes (emitted
    just before the last unit so no unit's a2a_in write queues behind it),
    the pass-2 stage is split in halves so its first accumulations start
    sooner.

Compute dtype bf16 (fp32 PSUM accumulation). Rel-l2 error vs the fp32
reference ~5.5e-3. HW exec ~210-233us (collective transfer time varies
15-38us run-to-run and dominates the remaining spread; the original
baseline measures 230-298us under the same conditions because its
gpsimd-blocked finalize stalls for the whole first AllToAll transfer).
Note: the final AllToAll's trigger is serialized ~19us behind the first
AllToAll's completion (CC turnaround), so the batch-1 thunk placement
(sb4-sb6 front-loaded into batch 0) balances trigger-0-earlier against
keeping enough batch-0 boundary filler; moving more thunks either way
measured worse.
"""

import numpy as np
import ml_dtypes

BF16 = ml_dtypes.bfloat16

B, S, E, H, D = 2, 2048, 1024, 16, 64
P = 128
SG = B * S          # 4096 global sequence length (batch-major)
NKO = E // P        # 8 contraction tiles over E
NST = SG // P       # 32 seq tiles of 128
NSB = SG // 512     # 8 seq blocks of 512
QB = S // 512       # 4 q-blocks per batch

_built = None
LAST_RESULTS = None


def _build():
    global _built
    if _built is not None:
        return _built

    import concourse.bacc as bacc
    import concourse.mybir as mybir
    import concourse.tile as tile

    f32 = mybir.dt.float32
    bf16 = mybir.dt.bfloat16
    Exp = mybir.ActivationFunctionType.Exp
    Identity = mybir.ActivationFunctionType.Identity

    nc = bacc.Bacc("TRN2", target_bir_lowering=False, debug=False, num_devices=8)

    # x pre-rearranged on host to [partition, seq-block, ko, 512] so each
    # chunk DMA moves 8KB-contiguous lines per partition on both sides
    xT = nc.declare_dram_parameter("xT", [P, NSB, NKO, 512], bf16, isOutput=False)
    wq = nc.declare_dram_parameter("wq", [E, P], bf16, isOutput=False)
    wk = nc.declare_dram_parameter("wk", [E, P], bf16, isOutput=False)
    wv = nc.declare_dram_parameter("wv", [E, P], bf16, isOutput=False)
    wo = nc.declare_dram_parameter("wo", [E, E], bf16, isOutput=False)
    bo = nc.declare_dram_parameter("bo", [P, NKO], f32, isOutput=False)
    masks = nc.declare_dram_parameter("masks", [P, P], bf16, isOutput=False)
    outT = nc.declare_dram_parameter("outT", [2, E, 512], bf16, isOutput=True)

    # AllToAll exchange, split by batch so the first one hides under compute:
    #   a2a_in0 chunk j (j<4) = my heads' attention for slot j   (batch 0)
    #   a2a_in1 chunk j (j>=4) = my heads' attention for slot j  (batch 1)
    # (unwritten chunks carry junk to cores that ignore them)
    # a2a_outG on core c holds rank r's heads for slot c in chunk r; it is
    # meaningful only on cores with c//4 == G.
    a2a_in0 = nc.dram_tensor("a2a_in0", [8, P, 512], bf16)
    a2a_in1 = nc.dram_tensor("a2a_in1", [8, P, 512], bf16)
    a2a_out0 = nc.dram_tensor("a2a_out0", [8, P, 512], bf16)
    a2a_out1 = nc.dram_tensor("a2a_out1", [8, P, 512], bf16)

    with tile.TileContext(nc) as tc, \
         tc.tile_pool(name="const", bufs=1) as const:
        # ---- constant / persistent SBUF tensors ----
        xT_sb = const.tile([P, NSB, NKO, 512], bf16, name="xT_sb")
        wq_sb = const.tile([P, NKO, P], bf16, name="wq_sb")
        wk_sb = const.tile([P, NKO, P], bf16, name="wk_sb")
        wv_sb = const.tile([P, NKO, P], bf16, name="wv_sb")
        wo_sb = const.tile([P, NKO, E], bf16, name="wo_sb")
        bo_sb = const.tile([P, NKO], f32, name="bo_sb")
        masks_sb = const.tile([P, P], bf16, name="masks_sb")
        qT_sb = const.tile([P, SG], bf16, name="qT_sb")
        kT_sb = const.tile([P, SG], bf16, name="kT_sb")
        # per seq-tile: [ones | v_h0(64) | ones | v_h1(64)] -- the leading ones
        # column makes the softmax denominator land on PSUM partition 0
        v_aug = const.tile([P, NST, 130], bf16, name="v_aug")
        # [1, 65] ones row: lhsT of the PE broadcast matmul that replicates
        # the softmax reciprocal across partitions (GpSimd must stay free:
        # it issues the collectives and blocks until they complete)
        ones_sb = const.tile([1, 65], bf16, name="ones_sb")
        attn_all0 = const.tile([P, 8, 512], bf16, name="attn_all0")
        attn_all1 = const.tile([P, 8, 512], bf16, name="attn_all1")
        out_sb0 = const.tile([P, NKO, 512], bf16, name="out_sb0")
        out_sb1 = const.tile([P, NKO, 512], bf16, name="out_sb1")

        # interleave weight loads with the first x chunks so the first
        # projection block (Q of seq-block 0) starts as early as possible;
        # first chunks split finely so the ko-serial projection can begin
        # after ~0.3MB of DMA
        wq_r = wq.rearrange("(ko p) m -> p ko m", p=P)
        nc.sync.dma_start(wq_sb[:, 0:2], wq_r[:, 0:2])
        nc.sync.dma_start(xT_sb[:, 0, 0:2], xT[:, 0, 0:2])
        nc.sync.dma_start(wq_sb[:, 2:8], wq_r[:, 2:8])
        nc.sync.dma_start(xT_sb[:, 0, 2:4], xT[:, 0, 2:4])
        nc.sync.dma_start(xT_sb[:, 0, 4:8], xT[:, 0, 4:8])
        nc.sync.dma_start(wk_sb, wk.rearrange("(ko p) m -> p ko m", p=P))
        nc.sync.dma_start(wv_sb, wv.rearrange("(ko p) m -> p ko m", p=P))
        nc.sync.dma_start(masks_sb, masks[:])
        for sb in (1, 2, 3):
            nc.sync.dma_start(xT_sb[:, sb, 0:4], xT[:, sb, 0:4])
            nc.sync.dma_start(xT_sb[:, sb, 4:8], xT[:, sb, 4:8])
        for sb in range(4, NSB):
            nc.sync.dma_start(xT_sb[:, sb], xT[:, sb])
        nc.sync.dma_start(bo_sb, bo[:])
        nc.sync.dma_start(wo_sb, wo.rearrange("(ko p) m -> p ko m", p=P))

        with tc.tile_pool(name="psBig", bufs=2, space="PSUM") as psBig, \
             tc.tile_pool(name="psSmall", bufs=4, space="PSUM") as psSmall, \
             tc.tile_pool(name="sb_att", bufs=3) as sba:
            nc.any.memset(v_aug[:, :, 0:1], 1.0)
            nc.any.memset(v_aug[:, :, 65:66], 1.0)
            nc.any.memset(ones_sb, 1.0)

            def proj_block(w_sb, dst, sb):
                ps = psBig.tile([P, 2, 512], f32, tag="big", name="ps_proj")
                for ko in range(NKO):
                    nc.tensor.matmul(
                        ps[:, 0, :],
                        w_sb[:, ko, :],
                        xT_sb[:, sb, ko, :],
                        start=(ko == 0),
                        stop=(ko == NKO - 1),
                    )
                nc.vector.tensor_copy(out=dst[:, sb * 512:(sb + 1) * 512], in_=ps[:, 0, :])

            def v_block(st):
                ps = psSmall.tile([P, P], f32, tag="small", name="ps_vproj")
                co = (st % 4) * P
                for ko in range(NKO):
                    nc.tensor.matmul(
                        ps,
                        xT_sb[:, st // 4, ko, co:co + P],
                        wv_sb[:, ko, :],
                        start=(ko == 0),
                        stop=(ko == NKO - 1),
                    )
                nc.vector.tensor_copy(
                    out=v_aug[:, st, 0:130].rearrange("p (h x) -> p h x", x=65)[:, :, 1:65],
                    in_=ps.rearrange("p (h x) -> p h x", x=64),
                )

            def attn_unit(b, qb, fin_prev=None):
                # software-pipelined: sc/exp of k-tile i+1 emitted before the
                # PV of k-tile i, so PE never stalls on ACT. Score tiles are
                # hl-paired [128(keys), 2(heads), 512(q)]. The PREVIOUS
                # unit's finalize is emitted after this unit's first PV so
                # its reciprocal chain hides under this unit's compute.
                numer = [
                    psSmall.tile([65, 512], f32, tag="small", name="ps_nm_t")
                    for _ in range(2)
                ]
                nkt = 4 * qb + 4
                q0 = S * b + qb * 512
                exs = {}

                def emit_sc_exp(kt):
                    sc = psBig.tile([P, 2, 512], f32, tag="big", name="ps_sc_t")
                    ex = sba.tile([P, 2, 512], bf16, tag="exp", name="sb_ex_t")
                    dj = kt - 4 * qb
                    qv = 128 * dj if dj > 0 else 0
                    for hl in range(2):
                        nc.tensor.matmul(
                            sc[:, hl, qv:512],
                            kT_sb[64 * hl:64 * hl + 64,
                                  S * b + kt * P:S * b + (kt + 1) * P],
                            qT_sb[64 * hl:64 * hl + 64, q0 + qv:q0 + 512],
                            start=True,
                            stop=True,
                        )
                    # exp only the q-range PV will read (strided over hl)
                    nc.scalar.activation(
                        ex[:, :, qv:512], sc[:, :, qv:512], Exp, scale=0.125
                    )
                    if dj >= 0:
                        # only the [128,128] diagonal square needs masking;
                        # leading q-columns are skipped by PV entirely
                        for hl in range(2):
                            nc.vector.tensor_mul(
                                out=ex[:, hl, 128 * dj:128 * dj + 128],
                                in0=ex[:, hl, 128 * dj:128 * dj + 128],
                                in1=masks_sb[:, 0:P],
                            )
                    exs[kt] = ex

                def emit_pv(kt):
                    dj = kt - 4 * qb
                    qv = 128 * dj if dj > 0 else 0
                    ex = exs.pop(kt)
                    for hl in range(2):
                        nc.tensor.matmul(
                            numer[hl][:, qv:512],
                            v_aug[:, 16 * b + kt, 65 * hl:65 * hl + 65],
                            ex[:, hl, qv:512],
                            start=(kt == 0),
                            stop=(kt == nkt - 1),
                        )

                emit_sc_exp(0)
                for kt in range(1, nkt):
                    emit_sc_exp(kt)
                    emit_pv(kt - 1)
                    if kt == 1 and fin_prev is not None:
                        fin_prev()
                emit_pv(nkt - 1)

                dst = a2a_in0 if b == 0 else a2a_in1
                slot = 4 * b + qb

                def finalize():
                    # numer is copied to SBUF first: that (not the final
                    # mul) releases the PSUM tile, so no later PE-queue
                    # matmul can wait on a free that itself needs a
                    # later-PE-queue instruction (deadlock-safe rotation)
                    for hl in range(2):
                        recip = sba.tile([1, 512], f32, tag="recip", name="sb_rc_t")
                        nc.vector.reciprocal_approx_fast(recip, numer[hl][0:1, :])
                        recip_b = sba.tile([1, 512], bf16, tag="recipb", name="sb_rcb_t")
                        nc.vector.tensor_copy(out=recip_b, in_=recip)
                        num_sb = sba.tile([65, 512], f32, tag="numcp", name="sb_nc_t")
                        nc.vector.tensor_copy(out=num_sb, in_=numer[hl][:, :])
                        rb = psSmall.tile([65, 512], f32, tag="small", name="ps_rb_t")
                        nc.tensor.matmul(rb, ones_sb, recip_b, start=True, stop=True)
                        attn = sba.tile([65, 512], bf16, tag="attn", name="sb_at_t")
                        nc.vector.tensor_mul(out=attn, in0=num_sb, in1=rb)
                        nc.sync.dma_start(
                            dst[slot, 64 * hl:64 * hl + 64, :], attn[1:65, :]
                        )
                return finalize

            def a2a(g):
                # NOTE: collectives must issue from gpsimd (the BIR
                # verifier rejects other engines), and the issuing engine
                # blocks until completion — so gpsimd carries no other work.
                src = a2a_in0 if g == 0 else a2a_in1
                dst = a2a_out0 if g == 0 else a2a_out1
                nc.gpsimd.collective_compute(
                    "AllToAll",
                    mybir.AluOpType.bypass,
                    replica_groups=[list(range(8))],
                    ins=[src[:].opt()],
                    outs=[dst[:].opt()],
                )

            def take(lst, n):
                for thunk in lst[:n]:
                    thunk()
                del lst[:n]

            # batch-0 prerequisites for unit (0,0) only; the rest of the
            # batch-0 projections are pipelined between batch-0 units
            proj_block(wq_sb, qT_sb, 0)
            proj_block(wk_sb, kT_sb, 0)
            for st in range(4):
                v_block(st)

            def sb_group(lst, sb):
                lst.append(lambda sb=sb: proj_block(wq_sb, qT_sb, sb))
                lst.append(lambda sb=sb: proj_block(wk_sb, kT_sb, sb))
                for st in range(4 * sb, 4 * sb + 4):
                    lst.append(lambda st=st: v_block(st))

            b0 = []
            for sb in range(1, 4):
                sb_group(b0, sb)

            # batch-1 projection thunks, grouped by seq-block (unit (1,j)
            # only needs block 4+j), spread across BOTH batches' attention
            # units so the ACT-bound batch-1 units keep the PE fed
            a1 = []
            for sb in range(4, 8):
                sb_group(a1, sb)

            fin = None
            for qb in range(QB):
                fin = attn_unit(0, qb, fin_prev=fin)
                take(b0, 6)        # next seq-block's q/k/v for batch 0
                # groups sb4-sb6 during batch 0: front-loading batch-1
                # projections moves the final AllToAll trigger earlier
                # (bounded by the first AllToAll still completing in time)
                if qb < 3:
                    take(a1, (4, 4, 5)[qb])
            fin()
            # the last thunk group is emitted AFTER the finalize so the
            # collective trigger isn't queued behind ~6us of projection
            # matmuls; these run during the rendezvous/transfer instead
            take(a1, 5)
            a2a(0)  # exchange batch-0 slots under batch-1 compute
            fin = None
            for qb in range(QB):
                fin = attn_unit(1, qb, fin_prev=fin)
                take(a1, (3, 3, 0, 0)[qb])  # group sb7 during (1,0),(1,1)
            # finalize FIRST so its a2a_in DMA dispatches (and thus the
            # final AllToAll trigger) aren't serialized behind the gather
            # dispatches on the sync engine
            fin()
            # stage the pass-1 input in 4 parallel-queue DMAs (one strided
            # DMA serializes into ~7us of descriptors on the pass-1
            # critical path). Emitted here: the first AllToAll has
            # completed by now, so these never block the sync queue.
            for ci in range(0, 8, 2):
                nc.sync.dma_start(
                    attn_all0[:, ci:ci + 2],
                    a2a_out0[ci:ci + 2].rearrange("c p f -> p c f"),
                )
            a2a(1)

            # ---- output projection, twice: pass g is valid on cores with
            # pid//4 == g; pass 1 overlaps the final AllToAll's transfer ----
            def outproj(attn_all, out_sb, outT_g):
                outT_r = outT_g.rearrange("(mo p) f -> p mo f", p=P)
                for mo in range(NKO):
                    ps = psBig.tile([P, 2, 512], f32, tag="big", name="ps_out")
                    for ci in range(8):
                        nc.tensor.matmul(
                            ps[:, 0, :],
                            wo_sb[:, ci, mo * P:(mo + 1) * P],
                            attn_all[:, ci, :],
                            start=(ci == 0),
                            stop=(ci == 7),
                        )
                    nc.scalar.activation(
                        out_sb[:, mo, :], ps[:, 0, :], Identity,
                        bias=bo_sb[:, mo:mo + 1], scale=1.0,
                    )
                    nc.sync.dma_start(
                        outT_r[:, mo:mo + 1, :], out_sb[:, mo:mo + 1, :]
                    )

            outproj(attn_all0, out_sb0, outT[0])
            # stage pass-2 input in two halves (parallel queues, and the
            # first accumulation matmuls only need chunks 0-3)
            nc.sync.dma_start(
                attn_all1[:, 0:4], a2a_out1[0:4].rearrange("c p f -> p c f")
            )
            nc.sync.dma_start(
                attn_all1[:, 4:8], a2a_out1[4:8].rearrange("c p f -> p c f")
            )
            outproj(attn_all1, out_sb1, outT[1])

    nc.compile()
    _built = nc
    return nc


def _host_masks():
    p = np.arange(P)[:, None]
    f = np.arange(P)[None, :]
    return np.ascontiguousarray((f >= p).astype(np.float32)).astype(BF16)


def kernel(**inputs):
    global LAST_RESULTS
    from concourse import bass_utils

    x = np.asarray(inputs["x"], np.float32)
    W_q = np.asarray(inputs["W_q"], np.float32)
    W_k = np.asarray(inputs["W_k"], np.float32)
    W_v = np.asarray(inputs["W_v"], np.float32)
    W_o = np.asarray(inputs["W_o"], np.float32)
    b_o = np.asarray(inputs["b_o"], np.float32)

    nc = _build()

    xT_all = np.concatenate([x[0].T, x[1].T], axis=1)   # [E, SG]
    # -> [partition, seq-block, ko, 512] (8KB-contiguous chunk lines)
    xT_all = np.ascontiguousarray(
        xT_all.reshape(NKO, P, NSB, 512).transpose(1, 2, 0, 3)
    ).astype(BF16)
    wo_b = np.ascontiguousarray(W_o).astype(BF16)
    bo_t = np.ascontiguousarray(b_o.reshape(NKO, P).T).astype(np.float32)
    masks = _host_masks()

    in_maps = []
    for c in range(8):
        sl = slice(P * c, P * (c + 1))
        in_maps.append({
            "xT": xT_all,
            "wq": np.ascontiguousarray(W_q[:, sl]).astype(BF16),
            "wk": np.ascontiguousarray(W_k[:, sl]).astype(BF16),
            "wv": np.ascontiguousarray(W_v[:, sl]).astype(BF16),
            "wo": wo_b,
            "bo": bo_t,
            "masks": masks,
        })

    res = bass_utils.run_bass_kernel_spmd(nc, in_maps, core_ids=list(range(8)))
    LAST_RESULTS = res

    out = np.empty((B, S, E), np.float32)
    for c in range(8):
        b, qb = c // 4, c % 4
        out[b, 512 * qb:512 * (qb + 1), :] = np.asarray(
            res.results[c]["outT"], np.float32
        )[c // 4].T
    return out.astype(np.float32)


# revision 60
# speedup vs baseline: 1.0812x; 1.0812x over previous
"""Masked multi-head attention (B=2, S=2048, E=1024, H=16, D=64) on 8 TRN2 cores.

Sharding: each core owns 2 heads (of 16) for BOTH batches.
  - QKV projections computed per-core for its 2 heads (transposed layouts).
    Batch-0 projections are software-pipelined INTO batch-0 attention (unit
    (0,qb) only needs seq-block qb), so attention starts right after the
    first 1.3MB of DMA instead of waiting for the full 8MB x load; batch-1
    projection blocks are spread across both batches' attention units.
  - Attention: flash-style with transposed scores (scoresT[k, q] tiles),
    software-pipelined at k-tile granularity (score matmuls of k-tile i+1
    emitted before the PV matmuls of k-tile i). Unsafe softmax (no max
    subtraction); denominator via a leading ones-column in the PV matmul;
    causal mask applied only on the [128,128] diagonal square; score
    matmuls AND exp AND PV all trimmed to the valid q-range of diagonal
    k-tiles. The per-unit softmax normalization broadcasts the reciprocal
    across partitions with a tiny contraction-1 PE matmul against a ones
    row (NOT gpsimd partition_broadcast: gpsimd issues the collectives and
    BLOCKS until they complete, so any gpsimd work queued after a trigger
    would stall the pipeline for the whole transfer). Each unit's finalize
    is emitted inside the next unit so its reciprocal chain hides under
    compute.
  - Exchange head-parallel -> slot-parallel via TWO AllToAlls into SEPARATE
    dram buffers: slots 0-3 (batch 0) exchanged right after batch-0
    attention (hidden under batch-1 compute); slots 4-7 at the end.
  - Output projection runs TWICE: pass 1 reads the batch-0 exchange buffer
    and runs DURING the final AllToAll's transfer window (the PE is
    otherwise idle there); pass 2 reads the batch-1 buffer after it lands.
    Pass g's result is valid on cores with pid//4 == g; both passes write
    their slice of outT[2, E, 512] and the host picks slice c//4 per core.
    Exchange buffers are staged to SBUF with one strided DMA each: the
    pass-1 stage is prefetched once the first AllToAll completes (emitted
    just before the last unit so no unit's a2a_in write queues behind it),
    the pass-2 stage is split in halves so its first accumulations start
    sooner.

Compute dtype bf16 (fp32 PSUM accumulation). Rel-l2 error vs the fp32
reference ~5.5e-3. HW exec ~210-233us (collective transfer time varies
15-38us run-to-run and dominates the remaining spread; the original
baseline measures 230-298us under the same conditions because its
gpsimd-blocked finalize stalls for the whole first AllToAll transfer).
Note: the final AllToAll's trigger is serialized ~19us behind the first
AllToAll's completion (CC turnaround), so the batch-1 thunk placement
(sb4-sb6 front-loaded into batch 0) balances trigger-0-earlier against
keeping enough batch-0 boundary filler; moving more thunks either way
measured worse.
"""

import numpy as np
import ml_dtypes

BF16 = ml_dtypes.bfloat16

B, S, E, H, D = 2, 2048, 1024, 16, 64
P = 128
SG = B * S          # 4096 global sequence length (batch-major)
NKO = E // P        # 8 contraction tiles over E
NST = SG // P       # 32 seq tiles of 128
NSB = SG // 512     # 8 seq blocks of 512
QB = S // 512       # 4 q-blocks per batch

_built = None
LAST_RESULTS = None


def _build():
    global _built
    if _built is not None:
        return _built

    import concourse.bacc as bacc
    import concourse.mybir as mybir
    import concourse.tile as tile

    f32 = mybir.dt.float32
    bf16 = mybir.dt.bfloat16
    Exp = mybir.ActivationFunctionType.Exp
    Identity = mybir.ActivationFunctionType.Identity

    nc = bacc.Bacc("TRN2", target_bir_lowering=False, debug=False, num_devices=8)

    # x pre-rearranged on host to [partition, seq-block, ko, 512] so each
    # chunk DMA moves 8KB-contiguous lines per partition on both sides
    xT = nc.declare_dram_parameter("xT", [P, NSB, NKO, 512], bf16, isOutput=False)
    wq = nc.declare_dram_parameter("wq", [E, P], bf16, isOutput=False)
    wk = nc.declare_dram_parameter("wk", [E, P], bf16, isOutput=False)
    wv = nc.declare_dram_parameter("wv", [E, P], bf16, isOutput=False)
    wo = nc.declare_dram_parameter("wo", [E, E], bf16, isOutput=False)
    bo = nc.declare_dram_parameter("bo", [P, NKO], f32, isOutput=False)
    masks = nc.declare_dram_parameter("masks", [P, P], bf16, isOutput=False)
    outT = nc.declare_dram_parameter("outT", [2, E, 512], bf16, isOutput=True)

    # AllToAll exchange, split by batch so the first one hides under compute:
    #   a2a_in0 chunk j (j<4) = my heads' attention for slot j   (batch 0)
    #   a2a_in1 chunk j (j>=4) = my heads' attention for slot j  (batch 1)
    # (unwritten chunks carry junk to cores that ignore them)
    # a2a_outG on core c holds rank r's heads for slot c in chunk r; it is
    # meaningful only on cores with c//4 == G.
    a2a_in0 = nc.dram_tensor("a2a_in0", [8, P, 512], bf16)
    a2a_in1 = nc.dram_tensor("a2a_in1", [8, P, 512], bf16)
    a2a_out0 = nc.dram_tensor("a2a_out0", [8, P, 512], bf16)
    a2a_out1 = nc.dram_tensor("a2a_out1", [8, P, 512], bf16)

    with tile.TileContext(nc) as tc, \
         tc.tile_pool(name="const", bufs=1) as const:
        # ---- constant / persistent SBUF tensors ----
        xT_sb = const.tile([P, NSB, NKO, 512], bf16, name="xT_sb")
        wq_sb = const.tile([P, NKO, P], bf16, name="wq_sb")
        wk_sb = const.tile([P, NKO, P], bf16, name="wk_sb")
        wv_sb = const.tile([P, NKO, P], bf16, name="wv_sb")
        wo_sb = const.tile([P, NKO, E], bf16, name="wo_sb")
        bo_sb = const.tile([P, NKO], f32, name="bo_sb")
        masks_sb = const.tile([P, P], bf16, name="masks_sb")
        qT_sb = const.tile([P, SG], bf16, name="qT_sb")
        kT_sb = const.tile([P, SG], bf16, name="kT_sb")
        # per seq-tile: [ones | v_h0(64) | ones | v_h1(64)] -- the leading ones
        # column makes the softmax denominator land on PSUM partition 0
        v_aug = const.tile([P, NST, 130], bf16, name="v_aug")
        # [1, 65] ones row: lhsT of the PE broadcast matmul that replicates
        # the softmax reciprocal across partitions (GpSimd must stay free:
        # it issues the collectives and blocks until they complete)
        ones_sb = const.tile([1, 65], bf16, name="ones_sb")
        attn_all0 = const.tile([P, 8, 512], bf16, name="attn_all0")
        attn_all1 = const.tile([P, 8, 512], bf16, name="attn_all1")
        out_sb0 = const.tile([P, NKO, 512], bf16, name="out_sb0")
        out_sb1 = const.tile([P, NKO, 512], bf16, name="out_sb1")

        # interleave weight loads with the first x chunks so the first
        # projection block (Q of seq-block 0) starts as early as possible;
        # first chunks split finely so the ko-serial projection can begin
        # after ~0.3MB of DMA
        wq_r = wq.rearrange("(ko p) m -> p ko m", p=P)
        nc.sync.dma_start(wq_sb[:, 0:2], wq_r[:, 0:2])
        nc.sync.dma_start(xT_sb[:, 0, 0:2], xT[:, 0, 0:2])
        nc.sync.dma_start(wq_sb[:, 2:8], wq_r[:, 2:8])
        nc.sync.dma_start(xT_sb[:, 0, 2:4], xT[:, 0, 2:4])
        nc.sync.dma_start(xT_sb[:, 0, 4:8], xT[:, 0, 4:8])
        nc.sync.dma_start(wk_sb, wk.rearrange("(ko p) m -> p ko m", p=P))
        nc.sync.dma_start(wv_sb, wv.rearrange("(ko p) m -> p ko m", p=P))
        nc.sync.dma_start(masks_sb, masks[:])
        for sb in (1, 2, 3):
            nc.sync.dma_start(xT_sb[:, sb, 0:4], xT[:, sb, 0:4])
            nc.sync.dma_start(xT_sb[:, sb, 4:8], xT[:, sb, 4:8])
        for sb in range(4, NSB):
            nc.sync.dma_start(xT_sb[:, sb], xT[:, sb])
        nc.sync.dma_start(bo_sb, bo[:])
        nc.sync.dma_start(wo_sb, wo.rearrange("(ko p) m -> p ko m", p=P))

        with tc.tile_pool(name="psBig", bufs=2, space="PSUM") as psBig, \
             tc.tile_pool(name="psSmall", bufs=4, space="PSUM") as psSmall, \
             tc.tile_pool(name="sb_att", bufs=3) as sba:
            nc.any.memset(v_aug[:, :, 0:1], 1.0)
            nc.any.memset(v_aug[:, :, 65:66], 1.0)
            nc.any.memset(ones_sb, 1.0)

            def proj_block(w_sb, dst, sb):
                ps = psBig.tile([P, 2, 512], f32, tag="big", name="ps_proj")
                for ko in range(NKO):
                    nc.tensor.matmul(
                        ps[:, 0, :],
                        w_sb[:, ko, :],
                        xT_sb[:, sb, ko, :],
                        start=(ko == 0),
                        stop=(ko == NKO - 1),
                    )
                nc.vector.tensor_copy(out=dst[:, sb * 512:(sb + 1) * 512], in_=ps[:, 0, :])

            def v_block(st):
                ps = psSmall.tile([P, P], f32, tag="small", name="ps_vproj")
                co = (st % 4) * P
                for ko in range(NKO):
                    nc.tensor.matmul(
                        ps,
                        xT_sb[:, st // 4, ko, co:co + P],
                        wv_sb[:, ko, :],
                        start=(ko == 0),
                        stop=(ko == NKO - 1),
                    )
                nc.vector.tensor_copy(
                    out=v_aug[:, st, 0:130].rearrange("p (h x) -> p h x", x=65)[:, :, 1:65],
                    in_=ps.rearrange("p (h x) -> p h x", x=64),
                )

            def attn_unit(b, qb, fin_prev=None):
                # software-pipelined: sc/exp of k-tile i+1 emitted before the
                # PV of k-tile i, so PE never stalls on ACT. Score tiles are
                # hl-paired [128(keys), 2(heads), 512(q)]. The PREVIOUS
                # unit's finalize is emitted after this unit's first PV so
                # its reciprocal chain hides under this unit's compute.
                numer = [
                    psSmall.tile([65, 512], f32, tag="small", name="ps_nm_t")
                    for _ in range(2)
                ]
                nkt = 4 * qb + 4
                q0 = S * b + qb * 512
                exs = {}

                def emit_sc_exp(kt):
                    sc = psBig.tile([P, 2, 512], f32, tag="big", name="ps_sc_t")
                    ex = sba.tile([P, 2, 512], bf16, tag="exp", name="sb_ex_t")
                    dj = kt - 4 * qb
                    qv = 128 * dj if dj > 0 else 0
                    for hl in range(2):
                        nc.tensor.matmul(
                            sc[:, hl, qv:512],
                            kT_sb[64 * hl:64 * hl + 64,
                                  S * b + kt * P:S * b + (kt + 1) * P],
                            qT_sb[64 * hl:64 * hl + 64, q0 + qv:q0 + 512],
                            start=True,
                            stop=True,
                        )
                    # exp only the q-range PV will read (strided over hl)
                    nc.scalar.activation(
                        ex[:, :, qv:512], sc[:, :, qv:512], Exp, scale=0.125
                    )
                    if dj >= 0:
                        # only the [128,128] diagonal square needs masking;
                        # leading q-columns are skipped by PV entirely
                        for hl in range(2):
                            nc.vector.tensor_mul(
                                out=ex[:, hl, 128 * dj:128 * dj + 128],
                                in0=ex[:, hl, 128 * dj:128 * dj + 128],
                                in1=masks_sb[:, 0:P],
                            )
                    exs[kt] = ex

                def emit_pv(kt):
                    dj = kt - 4 * qb
                    qv = 128 * dj if dj > 0 else 0
                    ex = exs.pop(kt)
                    for hl in range(2):
                        nc.tensor.matmul(
                            numer[hl][:, qv:512],
                            v_aug[:, 16 * b + kt, 65 * hl:65 * hl + 65],
                            ex[:, hl, qv:512],
                            start=(kt == 0),
                            stop=(kt == nkt - 1),
                        )

                emit_sc_exp(0)
                for kt in range(1, nkt):
                    emit_sc_exp(kt)
                    emit_pv(kt - 1)
                    if kt == 1 and fin_prev is not None:
                        fin_prev()
                emit_pv(nkt - 1)

                dst = a2a_in0 if b == 0 else a2a_in1
                slot = 4 * b + qb

                def finalize():
                    # numer is copied to SBUF first: that (not the final
                    # mul) releases the PSUM tile, so no later PE-queue
                    # matmul can wait on a free that itself needs a
                    # later-PE-queue instruction (deadlock-safe rotation)
                    for hl in range(2):
                        recip = sba.tile([1, 512], f32, tag="recip", name="sb_rc_t")
                        nc.vector.reciprocal_approx_fast(recip, numer[hl][0:1, :])
                        recip_b = sba.tile([1, 512], bf16, tag="recipb", name="sb_rcb_t")
                        nc.vector.tensor_copy(out=recip_b, in_=recip)
                        num_sb = sba.tile([65, 512], f32, tag="numcp", name="sb_nc_t")
                        nc.vector.tensor_copy(out=num_sb, in_=numer[hl][:, :])
                        rb = psSmall.tile([65, 512], f32, tag="small", name="ps_rb_t")
                        nc.tensor.matmul(rb, ones_sb, recip_b, start=True, stop=True)
                        attn = sba.tile([65, 512], bf16, tag="attn", name="sb_at_t")
                        nc.vector.tensor_mul(out=attn, in0=num_sb, in1=rb)
                        nc.sync.dma_start(
                            dst[slot, 64 * hl:64 * hl + 64, :], attn[1:65, :]
                        )
                return finalize

            def a2a(g):
                # NOTE: collectives must issue from gpsimd (the BIR
                # verifier rejects other engines), and the issuing engine
                # blocks until completion — so gpsimd carries no other work.
                src = a2a_in0 if g == 0 else a2a_in1
                dst = a2a_out0 if g == 0 else a2a_out1
                nc.gpsimd.collective_compute(
                    "AllToAll",
                    mybir.AluOpType.bypass,
                    replica_groups=[list(range(8))],
                    ins=[src[:].opt()],
                    outs=[dst[:].opt()],
                )

            def take(lst, n):
                for thunk in lst[:n]:
                    thunk()
                del lst[:n]

            # batch-0 prerequisites for unit (0,0) only; the rest of the
            # batch-0 projections are pipelined between batch-0 units
            proj_block(wq_sb, qT_sb, 0)
            proj_block(wk_sb, kT_sb, 0)
            for st in range(4):
                v_block(st)

            def sb_group(lst, sb):
                lst.append(lambda sb=sb: proj_block(wq_sb, qT_sb, sb))
                lst.append(lambda sb=sb: proj_block(wk_sb, kT_sb, sb))
                for st in range(4 * sb, 4 * sb + 4):
                    lst.append(lambda st=st: v_block(st))

            b0 = []
            for sb in range(1, 4):
                sb_group(b0, sb)

            # batch-1 projection thunks, grouped by seq-block (unit (1,j)
            # only needs block 4+j), spread across BOTH batches' attention
            # units so the ACT-bound batch-1 units keep the PE fed
            a1 = []
            for sb in range(4, 8):
                sb_group(a1, sb)

            fin = None
            for qb in range(QB):
                fin = attn_unit(0, qb, fin_prev=fin)
                take(b0, 6)        # next seq-block's q/k/v for batch 0
                # groups sb4-sb6 during batch 0: front-loading batch-1
                # projections moves the final AllToAll trigger earlier
                # (bounded by the first AllToAll still completing in time)
                take(a1, (4, 4, 5, 5)[qb])
            fin()
            a2a(0)  # exchange batch-0 slots under batch-1 compute
            fin = None
            for qb in range(QB):
                fin = attn_unit(1, qb, fin_prev=fin)
                take(a1, (3, 3, 0, 0)[qb])  # group sb7 during (1,0),(1,1)
            # stage the pass-1 input in 4 parallel-queue DMAs (one strided
            # DMA serializes into ~7us of descriptors on the pass-1
            # critical path). Emitted here: the first AllToAll has
            # completed by now, so these never block the sync queue.
            for ci in range(0, 8, 2):
                nc.sync.dma_start(
                    attn_all0[:, ci:ci + 2],
                    a2a_out0[ci:ci + 2].rearrange("c p f -> p c f"),
                )
            fin()
            a2a(1)

            # ---- output projection, twice: pass g is valid on cores with
            # pid//4 == g; pass 1 overlaps the final AllToAll's transfer ----
            def outproj(attn_all, out_sb, outT_g):
                outT_r = outT_g.rearrange("(mo p) f -> p mo f", p=P)
                for mo in range(NKO):
                    ps = psBig.tile([P, 2, 512], f32, tag="big", name="ps_out")
                    for ci in range(8):
                        nc.tensor.matmul(
                            ps[:, 0, :],
                            wo_sb[:, ci, mo * P:(mo + 1) * P],
                            attn_all[:, ci, :],
                            start=(ci == 0),
                            stop=(ci == 7),
                        )
                    nc.scalar.activation(
                        out_sb[:, mo, :], ps[:, 0, :], Identity,
                        bias=bo_sb[:, mo:mo + 1], scale=1.0,
                    )
                    nc.sync.dma_start(
                        outT_r[:, mo:mo + 1, :], out_sb[:, mo:mo + 1, :]
                    )

            outproj(attn_all0, out_sb0, outT[0])
            # stage pass-2 input in two halves (parallel queues, and the
            # first accumulation matmuls only need chunks 0-3)
            nc.sync.dma_start(
                attn_all1[:, 0:4], a2a_out1[0:4].rearrange("c p f -> p c f")
            )
            nc.sync.dma_start(
                attn_all1[:, 4:8], a2a_out1[4:8].rearrange("c p f -> p c f")
            )
            outproj(attn_all1, out_sb1, outT[1])

    nc.compile()
    _built = nc
    return nc


def _host_masks():
    p = np.arange(P)[:, None]
    f = np.arange(P)[None, :]
    return np.ascontiguousarray((f >= p).astype(np.float32)).astype(BF16)


def kernel(**inputs):
    global LAST_RESULTS
    from concourse import bass_utils

    x = np.asarray(inputs["x"], np.float32)
    W_q = np.asarray(inputs["W_q"], np.float32)
    W_k = np.asarray(inputs["W_k"], np.float32)
    W_v = np.asarray(inputs["W_v"], np.float32)
    W_o = np.asarray(inputs["W_o"], np.float32)
    b_o = np.asarray(inputs["b_o"], np.float32)

    nc = _build()

    xT_all = np.concatenate([x[0].T, x[1].T], axis=1)   # [E, SG]
    # -> [partition, seq-block, ko, 512] (8KB-contiguous chunk lines)
    xT_all = np.ascontiguousarray(
        xT_all.reshape(NKO, P, NSB, 512).transpose(1, 2, 0, 3)
    ).astype(BF16)
    wo_b = np.ascontiguousarray(W_o).astype(BF16)
    bo_t = np.ascontiguousarray(b_o.reshape(NKO, P).T).astype(np.float32)
    masks = _host_masks()

    in_maps = []
    for c in range(8):
        sl = slice(P * c, P * (c + 1))
        in_maps.append({
            "xT": xT_all,
            "wq": np.ascontiguousarray(W_q[:, sl]).astype(BF16),
            "wk": np.ascontiguousarray(W_k[:, sl]).astype(BF16),
            "wv": np.ascontiguousarray(W_v[:, sl]).astype(BF16),
            "wo": wo_b,
            "bo": bo_t,
            "masks": masks,
        })

    res = bass_utils.run_bass_kernel_spmd(nc, in_maps, core_ids=list(range(8)))
    LAST_RESULTS = res

    out = np.empty((B, S, E), np.float32)
    for c in range(8):
        b, qb = c // 4, c % 4
        out[b, 512 * qb:512 * (qb + 1), :] = np.asarray(
            res.results[c]["outT"], np.float32
        )[c // 4].T
    return out.astype(np.float32)
